# revision 13
# baseline (speedup 1.0000x reference)
"""Trainium2 Bass kernel for nn_Align (attention-pooling module).

reference:
    h = tanh(key^T W^T + b)        # [B,S,D] x [D,D]
    scores = h @ v                 # [B,S,1]
    p = softmax(scores, axis=S)
    x = sum(value * p, axis=S)     # [B,D]
    returns (x, p)

Strategy: data-parallel over batch across 8 cores (16 rows each), W
replicated. Host pre-transposes K and W so the contraction dim (d) lands
on SBUF partitions, and casts both to bf16 (TensorE runs bf16 at full
rate; fp32 is 4x slower). Everything else stays fp32; the tiny
scores/pooling matmuls use the f32r fast path.
"""

import numpy as np
import ml_dtypes

S, B, D = 128, 128, 2048
NCORES = 8
BL = B // NCORES      # batch rows per core = 16
NBLK = 4              # batch blocks per core
BPB = BL // NBLK      # batch rows per block = 4
CW = BPB * S          # columns per block = 512
DC = D // 128         # contraction chunks = 16
EC = D // 128         # output-dim chunks = 16
NQ = 4                # 512-wide quarters of the e axis

_CACHE = {}


def _build_graph():
    import concourse.bass as bass  # noqa: F401
    import concourse.mybir as mybir
    import concourse.tile as tile
    from concourse import bacc
    from concourse.masks import make_identity

    f32 = mybir.dt.float32
    bf16 = mybir.dt.bfloat16
    f32r = mybir.dt.float32r
    TANH = mybir.ActivationFunctionType.Tanh
    EXP = mybir.ActivationFunctionType.Exp
    AX = mybir.AxisListType.X

    nc = bacc.Bacc(None)
    kt_d = nc.declare_dram_parameter("kt", [NBLK, DC, 128, CW], bf16, isOutput=False)
    wt_d = nc.declare_dram_parameter("wt", [DC, 128, D], bf16, isOutput=False)
    bias_d = nc.declare_dram_parameter("bias", [128, EC], f32, isOutput=False)
    vt_d = nc.declare_dram_parameter("vt", [128, EC, BL], bf16, isOutput=False)
    vals_d = nc.declare_dram_parameter("vals", [BL, S, D], bf16, isOutput=False)
    mask_d = nc.declare_dram_parameter("mask", [BPB, BPB * S], f32, isOutput=False)
    x_d = nc.declare_dram_parameter("x", [BL, D], f32, isOutput=True)
    p_d = nc.declare_dram_parameter("p", [BL, S], f32, isOutput=True)

    with tile.TileContext(nc) as tc:
        import contextlib

        with contextlib.ExitStack() as ctx:
            const_pool = ctx.enter_context(tc.tile_pool(name="const", bufs=1))
            wt_pool = ctx.enter_context(tc.tile_pool(name="wt", bufs=1))
            kt_pool = ctx.enter_context(tc.tile_pool(name="kt", bufs=2))
            vals_pool = ctx.enter_context(tc.tile_pool(name="vals", bufs=2))
            h_pool = ctx.enter_context(tc.tile_pool(name="h", bufs=4))
            sc_pool = ctx.enter_context(tc.tile_pool(name="sc", bufs=2))
            small_pool = ctx.enter_context(tc.tile_pool(name="small", bufs=2))
            p_pool = ctx.enter_context(tc.tile_pool(name="p", bufs=2))
            pt_pool = ctx.enter_context(tc.tile_pool(name="pt", bufs=2))
            psum_h = ctx.enter_context(tc.tile_pool(name="psh", bufs=2, space="PSUM"))
            psum_sc = ctx.enter_context(tc.tile_pool(name="pssc", bufs=2, space="PSUM"))
            psum_pt = ctx.enter_context(tc.tile_pool(name="pspt", bufs=1, space="PSUM"))
            psum_x = ctx.enter_context(tc.tile_pool(name="psx", bufs=2, space="PSUM"))

            ident = const_pool.tile([BPB, BPB], f32)
            make_identity(nc, ident)
            bias_sb = const_pool.tile([128, EC], f32, tag="bias")
            nc.sync.dma_start(out=bias_sb, in_=bias_d[:, :])
            vt_sb = const_pool.tile([128, EC, BL], bf16, tag="vt")
            nc.sync.dma_start(out=vt_sb, in_=vt_d[:, :, :])
            mask_sb = const_pool.tile([BPB, BPB * S], f32, tag="mask")
            nc.sync.dma_start(out=mask_sb, in_=mask_d[:, :])

            # Weights: [128, 512] tiles per (dc, quarter); quarter 0 first so
            # the first ec-group's stationaries arrive before the rest.
            wt_sb = {}

            def dma_wt_quarter(q):
                for dc in range(DC):
                    t = wt_pool.tile([128, 512], bf16, tag=f"wt{dc}_{q}", name=f"wt{dc}_{q}")
                    nc.sync.dma_start(
                        out=t, in_=wt_d[dc][:, q * 512 : (q + 1) * 512]
                    )
                    wt_sb[dc, q] = t

            kt_sb = {}
            vals_sb = {}
            psc = {}
            hts = {}

            def dma_block_inputs(blk):
                t = kt_pool.tile([128, DC, CW], bf16, tag="kt", name="ktb")
                nc.sync.dma_start(
                    out=t, in_=kt_d[blk].rearrange("dc p c -> p dc c")
                )
                kt_sb[blk] = t
                for i in range(BPB):
                    vt_ = vals_pool.tile([S, D], bf16, tag=f"vals{i}", name=f"vals{i}")
                    nc.sync.dma_start(out=vt_, in_=vals_d[blk * BPB + i])
                    vals_sb[blk, i] = vt_

            def emit_main_group(blk, ec):
                ph = psum_h.tile([128, CW], f32, tag="ph")
                q, e0 = ec // 4, (ec % 4) * 128
                for dc in range(DC):
                    nc.tensor.matmul(
                        ph,
                        lhsT=wt_sb[dc, q][:, e0 : e0 + 128],
                        rhs=kt_sb[blk][:, dc, :],
                        start=(dc == 0),
                        stop=(dc == DC - 1),
                    )
                ht = h_pool.tile([128, CW], bf16, tag="hT")
                nc.scalar.activation(
                    out=ht, in_=ph, func=TANH, bias=bias_sb[:, ec : ec + 1], scale=1.0
                )
                hts[blk, ec] = ht

            def emit_score(blk, ec):
                nc.tensor.matmul(
                    psc[blk],
                    lhsT=vt_sb[:, ec, blk * BPB : (blk + 1) * BPB],
                    rhs=hts[blk, ec][:, :],
                    start=(ec == 0),
                    stop=(ec == EC - 1),
                    skip_group_check=True,
                )

            def emit_tail(blk):
                b0 = blk * BPB
                tmp = sc_pool.tile([BPB, BPB * S], f32, tag="tmp")
                nc.vector.tensor_mul(tmp, psc[blk], mask_sb)
                sc = sc_pool.tile([BPB, S], f32, tag="sc")
                nc.vector.reduce_sum(
                    out=sc, in_=tmp.rearrange("p (j s) -> p s j", j=BPB), axis=AX
                )
                mx = small_pool.tile([BPB, 1], f32, tag="mx")
                nc.vector.reduce_max(out=mx, in_=sc, axis=AX, negate=True)
                e4 = sc_pool.tile([BPB, S], f32, tag="e4")
                sm = small_pool.tile([BPB, 1], f32, tag="sm")
                nc.scalar.activation(
                    out=e4, in_=sc, func=EXP, bias=mx, scale=1.0, accum_out=sm
                )
                rec = small_pool.tile([BPB, 1], f32, tag="rec")
                nc.vector.reciprocal(out=rec, in_=sm)
                p4 = p_pool.tile([BPB, S], f32, tag="p4")
                nc.vector.tensor_scalar_mul(p4, e4, rec)
                nc.sync.dma_start(out=p_d[b0 : b0 + BPB, :], in_=p4)
                ppt = psum_pt.tile([S, BPB], f32, tag="ppt")
                nc.tensor.transpose(ppt, p4, ident)
                pt = pt_pool.tile([S, BPB], bf16, tag="pt")
                nc.vector.tensor_copy(out=pt, in_=ppt)
                for i in range(BPB):
                    for qq in range(NQ):
                        px = psum_x.tile([1, 512], f32, tag="px")
                        nc.tensor.matmul(
                            px,
                            lhsT=pt[:, i : i + 1],
                            rhs=vals_sb[blk, i][:, qq * 512 : (qq + 1) * 512],
                            start=True,
                            stop=True,
                        )
                        xq = p_pool.tile([1, 512], f32, tag="xq", name="xq", bufs=4)
                        nc.vector.tensor_copy(out=xq, in_=px)
                        nc.sync.dma_start(
                            out=x_d[b0 + i : b0 + i + 1, qq * 512 : (qq + 1) * 512],
                            in_=xq,
                        )

            # ---- emission schedule (software-pipelined tails) ----
            dma_wt_quarter(0)
            dma_block_inputs(0)
            for q in range(1, NQ):
                dma_wt_quarter(q)

            for blk in range(NBLK):
                psc[blk] = psum_sc.tile([BPB, CW], f32, tag="psc", name="psc")
                for ec in range(EC):
                    emit_main_group(blk, ec)
                    if ec == 0 and blk + 1 < NBLK:
                        dma_block_inputs(blk + 1)
                    if ec >= 1:
                        emit_score(blk, ec - 1)
                    if ec == 2 and blk > 0:
                        emit_tail(blk - 1)
                emit_score(blk, EC - 1)
            emit_tail(NBLK - 1)

    nc.finalize()
    return nc


def _get_compiled():
    if "nc" not in _CACHE:
        _CACHE["nc"] = _build_graph()
    return _CACHE["nc"]


def _prep_inputs(key, value, W, b, v):
    """Host-side shard + layout transform. Returns in_maps for 8 cores."""
    key = np.asarray(key, dtype=np.float32)
    value = np.asarray(value, dtype=np.float32)
    W = np.asarray(W, dtype=np.float32)
    b = np.asarray(b, dtype=np.float32)
    v = np.asarray(v, dtype=np.float32)

    WT = np.ascontiguousarray(W.T)  # [d, e]
    wt_h = WT.reshape(DC, 128, D).astype(ml_dtypes.bfloat16)
    bias_h = np.ascontiguousarray(b.reshape(EC, 128).T)  # [128, EC]
    mask_h = np.kron(np.eye(BPB, dtype=np.float32), np.ones(S, dtype=np.float32))

    in_maps = []
    for c in range(NCORES):
        bs = slice(c * BL, (c + 1) * BL)
        kc = key[:, bs, :]  # [S, BL, D]
        ktf = np.ascontiguousarray(kc.transpose(2, 1, 0)).reshape(D, BL * S)
        kt_h = np.ascontiguousarray(
            ktf.reshape(DC, 128, NBLK, CW).transpose(2, 0, 1, 3)
        ).astype(ml_dtypes.bfloat16)  # [blk, dc, p, c]
        vtc = v[bs, :, 0]  # [BL, D]
        vt_h = np.ascontiguousarray(vtc.reshape(BL, EC, 128).transpose(2, 1, 0)).astype(ml_dtypes.bfloat16)
        vals_h = np.ascontiguousarray(value[:, bs, :].transpose(1, 0, 2)).astype(ml_dtypes.bfloat16)  # [BL,S,D]
        in_maps.append(
            {
                "kt": kt_h,
                "wt": wt_h,
                "bias": bias_h,
                "vt": vt_h,
                "vals": vals_h,
                "mask": mask_h,
            }
        )
    return in_maps


def run(inputs, trace=False, tmpdir=None):
    """Run the kernel on 8 cores. Returns ((x, p_attn), exec_time_ns)."""
    from concourse.bass_utils import run_bass_kernel_spmd

    if trace:
        _install_profhook()

    nc = _get_compiled()
    in_maps = _prep_inputs(
        inputs["key"], inputs["value"], inputs["W"], inputs["b"], inputs["v"]
    )
    res = run_bass_kernel_spmd(
        nc, in_maps, core_ids=list(range(NCORES)), trace=trace, tmpdir=tmpdir
    )
    x_full = np.concatenate([np.asarray(res.results[c]["x"]) for c in range(NCORES)], 0)
    p_full = np.concatenate([np.asarray(res.results[c]["p"]) for c in range(NCORES)], 0)
    p_attn = p_full.reshape(B, S, 1).astype(np.float32)
    return (x_full.astype(np.float32), p_attn), res.exec_time_ns


def kernel(**inputs):
    out, _ = run(inputs, trace=False)
    return out


def _install_profhook():
    """Register the NTFF profiling hook that this image's antenv lacks."""
    import contextlib
    import ctypes
    import sys
    import types

    if "antenv.axon_hooks" in sys.modules:
        return
    so_path = "/opt/axon/libaxon_pjrt.so"
    lib = ctypes.CDLL(so_path)
    if not hasattr(lib, "axon_start_nrt_profile"):
        return
    lib.axon_start_nrt_profile.argtypes = [
        ctypes.POINTER(ctypes.c_int64),
        ctypes.c_size_t,
    ]
    lib.axon_start_nrt_profile.restype = ctypes.c_int64
    lib.axon_stop_nrt_profile.argtypes = [ctypes.c_char_p]
    lib.axon_stop_nrt_profile.restype = ctypes.c_int64

    @contextlib.contextmanager
    def _hook(output_dir, device_ids):
        import jax

        jax.devices()
        if device_ids:
            ids = (ctypes.c_int64 * len(device_ids))(*device_ids)
            rc = lib.axon_start_nrt_profile(ids, len(device_ids))
        else:
            rc = lib.axon_start_nrt_profile(None, 0)
        if rc != 0:
            raise RuntimeError(f"axon_start_nrt_profile rc={rc}")
        try:
            yield
        finally:
            n = lib.axon_stop_nrt_profile(str(output_dir).encode())
            if n < 0:
                raise RuntimeError(f"axon_stop_nrt_profile rc={n}")

    mod = types.ModuleType("antenv.axon_hooks")
    mod.get_axon_ntff_profile_hook = lambda: _hook
    mod.set_axon_ntff_profile_hook = lambda h: None
    import antenv

    antenv.axon_hooks = mod
    sys.modules["antenv.axon_hooks"] = mod

    from concourse import bass_utils

    bass_utils.upload_artifacts = lambda tmpdir: str(tmpdir)


# revision 30
# speedup vs baseline: 1.0472x; 1.0472x over previous
"""Trainium2 Bass kernel for nn_Align (attention-pooling module).

reference:
    h = tanh(key^T W^T + b)        # [B,S,D] x [D,D]
    scores = h @ v                 # [B,S,1]
    p = softmax(scores, axis=S)
    x = sum(value * p, axis=S)     # [B,D]
    returns (x, p)

Strategy: data-parallel over batch across 8 cores (16 rows each), W
replicated. Host pre-transposes K and W so the contraction dim (d) lands
on SBUF partitions, and casts matmul operands to bf16 (TensorE runs bf16
at 1 cycle/row; fp32 is 4x slower and f32r loses too much precision in
accumulation). PSUM accumulation is fp32 throughout; softmax and outputs
stay fp32. Per core: 4 batch-blocks of 4 rows, software-pipelined so the
scores matmuls, softmax, and weighted-sum pooling of block k overlap the
main matmuls of block k+1 on the PE.
"""

import numpy as np
import ml_dtypes

S, B, D = 128, 128, 2048
NCORES = 8
BL = B // NCORES      # batch rows per core = 16
NBLK = 4              # batch blocks per core
BPB = BL // NBLK      # batch rows per block = 4
CW = BPB * S          # columns per block = 512
DC = D // 128         # contraction chunks = 16
EC = D // 128         # output-dim chunks = 16
NQ = 4                # 512-wide quarters of the e axis

_CACHE = {}


def _build_graph(cfg=None):
    cfg = cfg or {}
    import concourse.bass as bass  # noqa: F401
    import concourse.mybir as mybir
    import concourse.tile as tile
    from concourse import bacc
    from concourse.masks import make_identity

    f32 = mybir.dt.float32
    bf16 = mybir.dt.bfloat16
    f32r = mybir.dt.float32r
    TANH = mybir.ActivationFunctionType.Tanh
    EXP = mybir.ActivationFunctionType.Exp
    AX = mybir.AxisListType.X

    nc = bacc.Bacc(None)
    kt_d = nc.declare_dram_parameter("kt", [NBLK, 128, DC, CW], bf16, isOutput=False)
    wt_d = nc.declare_dram_parameter("wt", [EC, 128, DC, 128], bf16, isOutput=False)
    bias_d = nc.declare_dram_parameter("bias", [128, EC], f32, isOutput=False)
    vt_d = nc.declare_dram_parameter("vt", [128, EC, BL], bf16, isOutput=False)
    vals_d = nc.declare_dram_parameter("vals", [BL, S, D], bf16, isOutput=False)
    mask_d = nc.declare_dram_parameter("mask", [BPB, BPB * S], f32, isOutput=False)
    x_d = nc.declare_dram_parameter("x", [BL, D], f32, isOutput=True)
    p_d = nc.declare_dram_parameter("p", [BL, S], f32, isOutput=True)

    with tile.TileContext(nc) as tc:
        import contextlib

        with contextlib.ExitStack() as ctx:
            const_pool = ctx.enter_context(tc.tile_pool(name="const", bufs=1))
            wt_pool = ctx.enter_context(tc.tile_pool(name="wt", bufs=1))
            kt_pool = ctx.enter_context(tc.tile_pool(name="kt", bufs=2))
            vals_pool = ctx.enter_context(tc.tile_pool(name="vals", bufs=2))
            h_pool = ctx.enter_context(tc.tile_pool(name="h", bufs=cfg.get("hbufs", 4)))
            sc_pool = ctx.enter_context(tc.tile_pool(name="sc", bufs=2))
            small_pool = ctx.enter_context(tc.tile_pool(name="small", bufs=2))
            p_pool = ctx.enter_context(tc.tile_pool(name="p", bufs=2))
            pt_pool = ctx.enter_context(tc.tile_pool(name="pt", bufs=2))
            psum_h = ctx.enter_context(tc.tile_pool(name="psh", bufs=cfg.get("psh", 2), space="PSUM"))
            psum_sc = ctx.enter_context(tc.tile_pool(name="pssc", bufs=2, space="PSUM"))
            psum_pt = ctx.enter_context(tc.tile_pool(name="pspt", bufs=1, space="PSUM"))
            psum_x = ctx.enter_context(tc.tile_pool(name="psx", bufs=cfg.get("psx", 3), space="PSUM"))

            ident = const_pool.tile([BPB, BPB], f32)
            make_identity(nc, ident)

            if cfg.get("warmup", False):
                # dummy matmuls during the initial DMA ramp: opens the PE HAM
                # clock gate (1.2 -> 2.4 GHz takes ~3.4us of sustained work)
                # before the first real matmul group arrives
                wu = const_pool.tile([128, 512], bf16, tag="wu")
                nc.vector.memset(wu, 0.0)
                for w_i in range(20):
                    pwu = psum_x.tile([128, 512], f32, tag="px", name="pwu")
                    nc.tensor.matmul(
                        pwu, lhsT=wu[:, :128], rhs=wu, start=True, stop=True
                    )
            bias_sb = const_pool.tile([128, EC], f32, tag="bias")
            nc.sync.dma_start(out=bias_sb, in_=bias_d[:, :])
            vt_sb = const_pool.tile([128, EC, BL], bf16, tag="vt")
            nc.sync.dma_start(out=vt_sb, in_=vt_d[:, :, :])
            mask_sb = const_pool.tile([BPB, BPB * S], f32, tag="mask")
            nc.sync.dma_start(out=mask_sb, in_=mask_d[:, :])

            # Weights: one [128, DC, 128] tile per output chunk ec;
            # contiguous 512KB DMA each.
            wt_sb = {}

            def dma_wt_ec(ec):
                t = wt_pool.tile(
                    [128, DC, 128], bf16, tag=f"wt{ec}", name=f"wt{ec}"
                )
                eng = nc.scalar if cfg.get("wt_on_act", False) else nc.sync
                eng.dma_start(out=t, in_=wt_d[ec])
                wt_sb[ec] = t

            kt_sb = {}
            vals_sb = {}
            psc = {}
            hts = {}

            def dma_block_inputs(blk, split=False):
                if split:
                    # block 0: per-dc tiles so the first MMs start as soon as
                    # their own slices land (kills the startup DMA bubble);
                    # weights for the next few ec groups ride along in
                    # consumption order
                    for dc in range(DC):
                        kt_t = kt_pool.tile(
                            [128, CW], bf16, tag=f"kt0_{dc}", name=f"kt0_{dc}", bufs=1
                        )
                        nc.sync.dma_start(out=kt_t, in_=kt_d[blk, :, dc, :])
                        kt_sb[blk, dc] = kt_t
                        if cfg.get("ilv", True) and dc % 4 == 3:
                            dma_wt_ec(1 + dc // 4)
                else:
                    kt_t = kt_pool.tile([128, DC, CW], bf16, tag="kt", name="ktb")
                    nc.sync.dma_start(out=kt_t, in_=kt_d[blk])
                    for dc in range(DC):
                        kt_sb[blk, dc] = kt_t[:, dc, :]
                for i in range(BPB):
                    vt_ = vals_pool.tile([S, D], bf16, tag=f"vals{i}", name=f"vals{i}")
                    nc.sync.dma_start(out=vt_, in_=vals_d[blk * BPB + i])
                    vals_sb[blk, i] = vt_

            def emit_main_group(blk, ec):
                ph = psum_h.tile([128, CW], f32, tag="ph")
                for dc in range(DC):
                    nc.tensor.matmul(
                        ph,
                        lhsT=wt_sb[ec][:, dc, :],
                        rhs=kt_sb[blk, dc],
                        start=(dc == 0),
                        stop=(dc == DC - 1),
                    )
                ht = h_pool.tile([128, CW], bf16, tag="hT")
                nc.scalar.activation(
                    out=ht, in_=ph, func=TANH, bias=bias_sb[:, ec : ec + 1], scale=1.0
                )
                hts[blk, ec] = ht

            def emit_score(blk, ec):
                nc.tensor.matmul(
                    psc[blk],
                    lhsT=vt_sb[:, ec, blk * BPB : (blk + 1) * BPB],
                    rhs=hts[blk, ec][:, :],
                    start=(ec == 0),
                    stop=(ec == EC - 1),
                    skip_group_check=True,
                )

            def emit_tail(blk):
                b0 = blk * BPB
                tmp = sc_pool.tile([BPB, BPB * S], f32, tag="tmp")
                nc.vector.tensor_mul(tmp, psc[blk], mask_sb)
                sc = sc_pool.tile([BPB, S], f32, tag="sc")
                nc.vector.reduce_sum(
                    out=sc, in_=tmp.rearrange("p (j s) -> p s j", j=BPB), axis=AX
                )
                mx = small_pool.tile([BPB, 1], f32, tag="mx")
                nc.vector.reduce_max(out=mx, in_=sc, axis=AX, negate=True)
                e4 = sc_pool.tile([BPB, S], f32, tag="e4")
                sm = small_pool.tile([BPB, 1], f32, tag="sm")
                nc.scalar.activation(
                    out=e4, in_=sc, func=EXP, bias=mx, scale=1.0, accum_out=sm
                )
                rec = small_pool.tile([BPB, 1], f32, tag="rec")
                nc.vector.reciprocal(out=rec, in_=sm)
                p4 = p_pool.tile([BPB, S], f32, tag="p4")
                nc.vector.tensor_scalar_mul(p4, e4, rec)
                nc.sync.dma_start(out=p_d[b0 : b0 + BPB, :], in_=p4)
                ppt = psum_pt.tile([S, BPB], f32, tag="ppt")
                nc.tensor.transpose(ppt, p4, ident)
                pt = pt_pool.tile([S, BPB], bf16, tag="pt")
                nc.vector.tensor_copy(out=pt, in_=ppt)
                for i in range(BPB):
                    for qq in range(NQ):
                        px = psum_x.tile([1, 512], f32, tag="px")
                        nc.tensor.matmul(
                            px,
                            lhsT=pt[:, i : i + 1],
                            rhs=vals_sb[blk, i][:, qq * 512 : (qq + 1) * 512],
                            start=True,
                            stop=True,
                        )
                        xq = p_pool.tile([1, 512], f32, tag="xq", name="xq", bufs=8)
                        if qq % 2 == 0:
                            nc.vector.tensor_copy(out=xq, in_=px)
                        else:
                            nc.scalar.copy(out=xq, in_=px)
                        nc.sync.dma_start(
                            out=x_d[b0 + i : b0 + i + 1, qq * 512 : (qq + 1) * 512],
                            in_=xq,
                        )

            # ---- emission schedule (software-pipelined tails) ----
            dma_wt_ec(0)
            dma_block_inputs(0, split=True)
            for ec in range(5 if cfg.get("ilv", True) else 1, EC):
                dma_wt_ec(ec)

            for blk in range(NBLK):
                psc[blk] = psum_sc.tile([BPB, CW], f32, tag="psc", name="psc")
                for ec in range(EC):
                    emit_main_group(blk, ec)
                    if ec == 0 and blk + 1 < NBLK:
                        dma_block_inputs(blk + 1)
                    if ec >= 1:
                        emit_score(blk, ec - 1)
                    if ec == 2 and blk > 0:
                        emit_tail(blk - 1)
                emit_score(blk, EC - 1)
            emit_tail(NBLK - 1)

    nc.finalize()
    return nc


def _get_compiled():
    if "nc" not in _CACHE:
        _CACHE["nc"] = _build_graph()
    return _CACHE["nc"]


def _prep_inputs(key, value, W, b, v):
    """Host-side shard + layout transform. Returns in_maps for 8 cores."""
    key = np.asarray(key, dtype=np.float32)
    value = np.asarray(value, dtype=np.float32)
    W = np.asarray(W, dtype=np.float32)
    b = np.asarray(b, dtype=np.float32)
    v = np.asarray(v, dtype=np.float32)

    WT = np.ascontiguousarray(W.T)  # [d, e]
    wt_h = np.ascontiguousarray(
        WT.reshape(DC, 128, EC, 128).transpose(2, 1, 0, 3)
    ).astype(ml_dtypes.bfloat16)  # [ec, p, dc, e_l]
    bias_h = np.ascontiguousarray(b.reshape(EC, 128).T)  # [128, EC]
    mask_h = np.kron(np.eye(BPB, dtype=np.float32), np.ones(S, dtype=np.float32))

    in_maps = []
    for c in range(NCORES):
        bs = slice(c * BL, (c + 1) * BL)
        kc = key[:, bs, :]  # [S, BL, D]
        ktf = np.ascontiguousarray(kc.transpose(2, 1, 0)).reshape(D, BL * S)
        kt_h = np.ascontiguousarray(
            ktf.reshape(DC, 128, NBLK, CW).transpose(2, 1, 0, 3)
        ).astype(ml_dtypes.bfloat16)  # [blk, p, dc, c]
        vtc = v[bs, :, 0]  # [BL, D]
        vt_h = np.ascontiguousarray(vtc.reshape(BL, EC, 128).transpose(2, 1, 0)).astype(ml_dtypes.bfloat16)
        vals_h = np.ascontiguousarray(value[:, bs, :].transpose(1, 0, 2)).astype(ml_dtypes.bfloat16)  # [BL,S,D]
        in_maps.append(
            {
                "kt": kt_h,
                "wt": wt_h,
                "bias": bias_h,
                "vt": vt_h,
                "vals": vals_h,
                "mask": mask_h,
            }
        )
    return in_maps


def run(inputs, trace=False, tmpdir=None):
    """Run the kernel on 8 cores. Returns ((x, p_attn), exec_time_ns)."""
    from concourse.bass_utils import run_bass_kernel_spmd

    if trace:
        _install_profhook()

    nc = _get_compiled()
    in_maps = _prep_inputs(
        inputs["key"], inputs["value"], inputs["W"], inputs["b"], inputs["v"]
    )
    res = run_bass_kernel_spmd(
        nc, in_maps, core_ids=list(range(NCORES)), trace=trace, tmpdir=tmpdir
    )
    x_full = np.concatenate([np.asarray(res.results[c]["x"]) for c in range(NCORES)], 0)
    p_full = np.concatenate([np.asarray(res.results[c]["p"]) for c in range(NCORES)], 0)
    p_attn = p_full.reshape(B, S, 1).astype(np.float32)
    return (x_full.astype(np.float32), p_attn), res.exec_time_ns


def kernel(**inputs):
    out, _ = run(inputs, trace=False)
    return out


def _install_profhook():
    """Register the NTFF profiling hook that this image's antenv lacks."""
    import contextlib
    import ctypes
    import sys
    import types

    if "antenv.axon_hooks" in sys.modules:
        return
    so_path = "/opt/axon/libaxon_pjrt.so"
    lib = ctypes.CDLL(so_path)
    if not hasattr(lib, "axon_start_nrt_profile"):
        return
    lib.axon_start_nrt_profile.argtypes = [
        ctypes.POINTER(ctypes.c_int64),
        ctypes.c_size_t,
    ]
    lib.axon_start_nrt_profile.restype = ctypes.c_int64
    lib.axon_stop_nrt_profile.argtypes = [ctypes.c_char_p]
    lib.axon_stop_nrt_profile.restype = ctypes.c_int64

    @contextlib.contextmanager
    def _hook(output_dir, device_ids):
        import jax

        jax.devices()
        if device_ids:
            ids = (ctypes.c_int64 * len(device_ids))(*device_ids)
            rc = lib.axon_start_nrt_profile(ids, len(device_ids))
        else:
            rc = lib.axon_start_nrt_profile(None, 0)
        if rc != 0:
            raise RuntimeError(f"axon_start_nrt_profile rc={rc}")
        try:
            yield
        finally:
            n = lib.axon_stop_nrt_profile(str(output_dir).encode())
            if n < 0:
                raise RuntimeError(f"axon_stop_nrt_profile rc={n}")

    mod = types.ModuleType("antenv.axon_hooks")
    mod.get_axon_ntff_profile_hook = lambda: _hook
    mod.set_axon_ntff_profile_hook = lambda h: None
    import antenv

    antenv.axon_hooks = mod
    sys.modules["antenv.axon_hooks"] = mod

    from concourse import bass_utils

    bass_utils.upload_artifacts = lambda tmpdir: str(tmpdir)
